# revision 15
# baseline (speedup 1.0000x reference)
"""Trainium2 Bass kernel for nn_Evaluator_40870908788848 (contour-weighted loss map).

Math (matches reference.py exactly in fp32):
  fw = sum_c target[c];  b = fw > 0
  contour = max over 9x9-window *differing* neighbors of r(dy,dx),
            r = 1/(k+1e-10)  (equivalent to the reference's 1/(min_k + 1e-10))
  out = minmax_norm((fw + contour)^2) * b     (min/max global over HxW)

Device mapping (rows sharded 256/core, 4-row halo pre-padded by host):
  - The two "differing" sides (b=0: neighbors with b=1; b=1: neighbors with
    b=0) are convolutions of b resp. (1-b) with fixed tap weights -> PE-array
    banded-Toeplitz matmuls (row shifts in the stationary, column shifts in
    the moving operand's free-axis offset).  128-row interior tiles; the last
    8 output rows' cross-tile taps accumulate via a second small stationary
    reading the next row-tile, into the same PSUM bank.
  - Tap weights give each distinct r its own power-of-two exponent digit
    (descending r), so the conv sum's magnitude identifies max-r-present; a
    16-step tensor_scalar(is_ge,mult)+tensor_tensor(max) sweep decodes it
    exactly in fp32.
  - Global min/max: per-core reduce, one AllReduce(max) of [max(w), max(-w)],
    normalize + mask on device.
"""
import math
import sys

sys.path.insert(0, "/opt/trn_rl_repo")

import numpy as np

import concourse.bass as bass
import concourse.mybir as mybir
import concourse.tile as tile
import concourse.bacc as bacc
from concourse.bass_utils import run_bass_kernel_spmd

F32 = mybir.dt.float32
BF16 = mybir.dt.bfloat16
U8 = mybir.dt.uint8
OP = mybir.AluOpType
AX = mybir.AxisListType
NP_BF16 = mybir.dt.np(BF16)

H, W, C = 2048, 2048, 5
NCORES = 8
PAD = 4
K = 9
TILE = 128
COL = 512


def _plan_taps(kern):
    """Distinct r=1/(k+1e-10) descending; assign exponent digit weights."""
    kern = np.asarray(kern, np.float32)
    taps = []
    for i in range(K):
        for j in range(K):
            if i == PAD and j == PAD:
                continue
            r = np.float32(np.float32(1.0) / (kern[i, j] + np.float32(1e-10)))
            taps.append((r, i - PAD, j - PAD))
    vals = sorted({t[0] for t in taps}, reverse=True)
    chunks, cur, cur_bits, groups = [], [], 0, []
    for v in vals:
        offs = [(dy, dx) for (r, dy, dx) in taps if r == v]
        bits = max(2, math.ceil(math.log2(len(offs) + 1)) + 1)
        if cur_bits + bits > 120:
            chunks.append(cur)
            cur, cur_bits = [], 0
        w = np.float32(2.0 ** (-(cur_bits + bits)))
        cur_bits += bits
        cur.append(len(groups))
        groups.append((np.float32(v), w, offs))
    if cur:
        chunks.append(cur)
    return chunks, groups


def _stationaries(chunks, groups, n_out, tail):
    """Banded conv stationaries.

    amatA[ci,dx][p, y] = tap weight (dy=p-y-PAD) for same-tile rows;
    amatB[ci,dx][q, y] = tap weight (dy=q+TILE-PAD-y) for next-tile rows
    (used when n_out == TILE).  sel picks fw at the interior alignment.
    """
    nch = len(chunks)
    amatA = np.zeros((nch, K, 128, n_out), np.float32)
    amatB = np.zeros((nch, K, 128, n_out), np.float32)
    for ci, gids in enumerate(chunks):
        for gi in gids:
            _, wgt, offs = groups[gi]
            for (dy, dx) in offs:
                j = dx + PAD
                for y in range(n_out):
                    p = y + PAD + dy
                    if p < TILE:
                        amatA[ci, j, p, y] = wgt
                    elif tail:
                        amatB[ci, j, p - TILE, y] = wgt
    selA = np.zeros((128, n_out), np.float32)
    selB = np.zeros((128, n_out), np.float32)
    for y in range(n_out):
        p = y + PAD
        if p < TILE:
            selA[p, y] = 1.0
        elif tail:
            selB[p - TILE, y] = 1.0
    return amatA, amatB, selA, selB


def build_nc(chunks, groups, rows_per_core, width, n_chan, reps=1, no_collective=False):
    nch = len(chunks)
    n_rt = (rows_per_core + TILE - 1) // TILE
    col = min(COL, width)
    n_ct = (width + col - 1) // col
    tail = rows_per_core % TILE == 0
    pr = rows_per_core + 2 * PAD
    bt_starts = list(range(0, pr, TILE))

    nc = bacc.Bacc("TRN2", target_bir_lowering=False, debug=False)
    tgt = nc.declare_dram_parameter("tgt", [n_chan, pr, width], BF16, isOutput=False)
    amatA_d = nc.declare_dram_parameter("amatA", [128, nch * K * TILE], BF16, isOutput=False)
    sel_d = nc.declare_dram_parameter("sel", [128, 2 * TILE], BF16, isOutput=False)
    if tail:
        amatB_d = nc.declare_dram_parameter("amatB", [128, nch * K * TILE], BF16, isOutput=False)
    out_d = nc.declare_dram_parameter("out", [rows_per_core, width], F32, isOutput=True)
    scr = nc.dram_tensor("scr", [2, 128], F32)
    cc_in = nc.dram_tensor("cc_in", [1, 2], F32)
    cc_out = nc.dram_tensor("cc_out", [1, 2], F32)

    wh = width + 2 * PAD

    with tile.TileContext(nc) as tc:
        with tc.tile_pool(name="const", bufs=1) as cst, \
             tc.tile_pool(name="chp", bufs=2) as chp, \
             tc.tile_pool(name="fwp", bufs=2) as fwp, \
             tc.tile_pool(name="bt", bufs=1) as btp, \
             tc.tile_pool(name="sdp", bufs=1) as sdp, \
             tc.tile_pool(name="accp", bufs=1) as accp, \
             tc.tile_pool(name="stash", bufs=1) as stash, \
             tc.tile_pool(name="stat", bufs=1) as stat, \
             tc.tile_pool(name="ps", bufs=2, space="PSUM") as ps:

            aA_t = cst.tile([128, nch * K * TILE], BF16, tag="amatA")
            sel_t = cst.tile([128, 2 * TILE], BF16, tag="sel")
            nc.sync.dma_start(aA_t[:], amatA_d[:])
            nc.sync.dma_start(sel_t[:], sel_d[:])
            if tail:
                aB_t = cst.tile([128, nch * K * TILE], BF16, tag="amatB")
                nc.sync.dma_start(aB_t[:], amatB_d[:])

            import contextlib
            loop_cm = tc.For_i(0, reps, 1) if reps > 1 else contextlib.nullcontext()
            loop_cm.__enter__()
            gmx = stat.tile([128, 1], F32, tag="gmx")
            gnm = stat.tile([128, 1], F32, tag="gnm")
            nc.vector.memset(gmx[:], -3.0e38)
            nc.vector.memset(gnm[:], -3.0e38)

            # ---- stage 1: fw / b / bbar for every b-tile (all bf16; exact for
            # binary inputs: sums are small integers)
            b_l, bb_l, fwb_l = [], [], []
            for k, s in enumerate(bt_starts):
                L = min(TILE, pr - s)
                ch = [chp.tile([128, width], BF16, tag=f"ch{i}", name=f"ch{i}") for i in range(2)]
                fw = btp.tile([128, width], BF16, tag=f"fw{k}", name=f"fw{k}")
                nc.sync.dma_start(ch[0][:L, :], tgt[0, s:s + L, :])
                nc.sync.dma_start(ch[1][:L, :], tgt[1, s:s + L, :])
                nc.vector.tensor_tensor(fw[:L, :], ch[0][:L, :], ch[1][:L, :], OP.add)
                for cidx in range(2, n_chan):
                    cht = chp.tile([128, width], BF16, tag=f"ch{cidx % 2}")
                    nc.sync.dma_start(cht[:L, :], tgt[cidx, s:s + L, :])
                    nc.vector.tensor_tensor(fw[:L, :], fw[:L, :], cht[:L, :], OP.add)
                b_t = btp.tile([128, wh], BF16, tag=f"b{k}", name=f"b{k}")
                bb_t = btp.tile([128, wh], BF16, tag=f"bb{k}", name=f"bb{k}")
                nc.vector.tensor_scalar(b_t[:L, PAD:PAD + width], fw[:L, :], 0.0, None, OP.is_gt)
                nc.vector.tensor_scalar(bb_t[:L, PAD:PAD + width], fw[:L, :], 0.0, None, OP.is_le)
                for e in range(PAD):
                    for tt in (b_t, bb_t):
                        nc.vector.tensor_copy(tt[:L, e:e + 1], tt[:L, PAD:PAD + 1])
                        nc.vector.tensor_copy(
                            tt[:L, PAD + width + e:PAD + width + e + 1],
                            tt[:L, PAD + width - 1:PAD + width])
                b_l.append(b_t); bb_l.append(bb_t); fwb_l.append(fw)

            # ---- stage 2: conv + decode per interior row-tile
            wst_l, mst_l, nt_l = [], [], []
            for t in range(n_rt):
                r0 = t * TILE
                nt = min(TILE, rows_per_core - r0)
                Lt = min(TILE, pr - bt_starts[t])
                use_tail = tail and nt == TILE

                sdiff = [sdp.tile([128, width], F32, tag=f"sd{ci}", name=f"sd{ci}") for ci in range(nch)]
                fwis = fwp.tile([128, width], F32, tag="fwis")
                mask_t = stash.tile([128, width], U8, tag=f"mask{t}", name=f"mask{t}")

                for cc in range(n_ct):
                    c0 = cc * col
                    cw = min(col, width - c0)
                    fwi_p = ps.tile([128, col], F32, tag="fwi")
                    nc.tensor.matmul(fwi_p[:nt, :cw], sel_t[:Lt, 0:nt],
                                     fwb_l[t][:Lt, c0:c0 + cw],
                                     start=True, stop=not use_tail)
                    if use_tail:
                        nc.tensor.matmul(fwi_p[:nt, :cw], sel_t[0:8, TILE:TILE + nt],
                                         fwb_l[t + 1][0:8, c0:c0 + cw],
                                         start=False, stop=True)
                    nc.scalar.copy(fwis[:nt, c0:c0 + cw], fwi_p[:nt, :cw])
                    nc.vector.tensor_scalar(
                        mask_t[:nt, c0:c0 + cw], fwis[:nt, c0:c0 + cw], 0.0, None, OP.is_gt)
                    for ci in range(nch):
                        s1_p = ps.tile([128, col], F32, tag="s1")
                        s0_p = ps.tile([128, col], F32, tag="s0")
                        for mov, psum in ((b_l, s1_p), (bb_l, s0_p)):
                            for dx in range(K):
                                a_ap = aA_t[:Lt, (ci * K + dx) * TILE:(ci * K + dx) * TILE + nt]
                                nc.tensor.matmul(psum[:nt, :cw], a_ap,
                                                 mov[t][:Lt, c0 + dx:c0 + dx + cw],
                                                 start=(dx == 0),
                                                 stop=(dx == K - 1 and not use_tail))
                            if use_tail:
                                for dx in range(K):
                                    a_ap = aB_t[0:8, (ci * K + dx) * TILE:(ci * K + dx) * TILE + nt]
                                    nc.tensor.matmul(psum[:nt, :cw], a_ap,
                                                     mov[t + 1][0:8, c0 + dx:c0 + dx + cw],
                                                     start=False, stop=(dx == K - 1))
                        nc.vector.tensor_copy(sdiff[ci][:nt, c0:c0 + cw], s1_p[:nt, :cw])
                        nc.vector.copy_predicated(
                            sdiff[ci][:nt, c0:c0 + cw], mask_t[:nt, c0:c0 + cw],
                            s0_p[:nt, :cw])

                acc = accp.tile([128, width], F32, tag="acc")
                first = True
                for ci, gids in enumerate(chunks):
                    for gi in gids:
                        rv, wgt, _ = groups[gi]
                        dst = acc if first else accp.tile([128, width], F32, tag="cand", name="cand")
                        nc.vector.tensor_scalar(
                            dst[:nt, :], sdiff[ci][:nt, :], float(wgt), float(rv),
                            OP.is_ge, OP.mult)
                        if not first:
                            nc.vector.tensor_tensor(acc[:nt, :], acc[:nt, :], dst[:nt, :], OP.max)
                        first = False

                wt = stash.tile([128, width], F32, tag=f"w{t}", name=f"w{t}")
                nc.vector.tensor_tensor(acc[:nt, :], acc[:nt, :], fwis[:nt, :], OP.add)
                nc.scalar.square(wt[:nt, :], acc[:nt, :])
                lred = stat.tile([128, 2], F32, tag="lred")
                nc.vector.tensor_reduce(lred[:nt, 0:1], wt[:nt, :], AX.X, OP.max)
                nc.vector.tensor_reduce(lred[:nt, 1:2], wt[:nt, :], AX.X, OP.min)
                nc.vector.tensor_tensor(gmx[:nt, :], gmx[:nt, :], lred[:nt, 0:1], OP.max)
                nc.vector.scalar_tensor_tensor(
                    gnm[:nt, :], lred[:nt, 1:2], -1.0, gnm[:nt, :], OP.mult, OP.max)
                wst_l.append(wt); mst_l.append(mask_t); nt_l.append(nt)

            loop_cm.__exit__(None, None, None)
            # ---- global min/max via DMA round-trip + AllReduce(max)
            nc.sync.dma_start(scr[0, :], gmx[:, 0:1])
            nc.sync.dma_start(scr[1, :], gnm[:, 0:1])
            srow = stat.tile([1, 2, 128], F32, tag="srow")
            nc.sync.dma_start(srow[0:1, :, :], scr[:, :])
            loc = stat.tile([1, 2], F32, tag="loc")
            nc.vector.tensor_reduce(loc[0:1, :], srow[0:1, :, :], AX.X, OP.max)
            nc.sync.dma_start(cc_in[:], loc[0:1, :])
            if no_collective:
                nc.sync.dma_start(cc_out[:], cc_in[:])
            else:
                nc.gpsimd.collective_compute(
                    "AllReduce", OP.max, replica_groups=[list(range(NCORES))],
                    ins=[cc_in[:]], outs=[cc_out[:]])
            st = stat.tile([1, 2], F32, tag="st")
            nc.sync.dma_start(st[:], cc_out[:])
            bcs = stat.tile([1, 2], F32, tag="bcs")
            rngt = stat.tile([1, 1], F32, tag="rngt")
            nc.vector.tensor_tensor(rngt[0:1, :], st[0:1, 0:1], st[0:1, 1:2], OP.add)
            nc.vector.tensor_scalar(rngt[0:1, :], rngt[0:1, :], 1e-10, None, OP.add)
            nc.vector.reciprocal(bcs[0:1, 0:1], rngt[0:1, :])
            mnt = stat.tile([1, 1], F32, tag="mnt")
            nc.vector.tensor_scalar(mnt[0:1, :], st[0:1, 1:2], -1.0, None, OP.mult)
            nc.vector.tensor_tensor(bcs[0:1, 1:2], mnt[0:1, :], bcs[0:1, 0:1], OP.mult)
            ones1 = stat.tile([1, 128], F32, tag="ones1")
            nc.vector.memset(ones1[:], 1.0)
            pbc = ps.tile([128, 2], F32, tag="pbc")
            nc.tensor.matmul(pbc[:], ones1[0:1, :], bcs[0:1, :], start=True, stop=True)
            bct = stat.tile([128, 2], F32, tag="bct")
            nc.vector.tensor_copy(bct[:], pbc[:])

            # ---- pass 2: normalize + mask + store
            for t in range(n_rt):
                nt = nt_l[t]
                r0 = t * TILE
                ot = accp.tile([128, width], F32, tag="cand", name="ot")
                nc.vector.tensor_scalar(
                    ot[:nt, :], wst_l[t][:nt, :], bct[:nt, 0:1], bct[:nt, 1:2],
                    OP.mult, OP.subtract)
                nc.vector.tensor_tensor(ot[:nt, :], ot[:nt, :], mst_l[t][:nt, :], OP.mult)
                nc.sync.dma_start(out_d[r0:r0 + nt, :], ot[:nt, :])
    nc.finalize()
    return nc


def _prep_inputs(target, kern, rows_per_core, width, n_chan):
    chunks, groups = _plan_taps(kern)
    nch = len(chunks)
    tail = rows_per_core % TILE == 0
    n_out = min(TILE, rows_per_core)
    amatA, amatB, selA, selB = _stationaries(chunks, groups, n_out, tail)

    def pack(am):
        p = np.zeros((128, nch * K * TILE), np.float32)
        for ci in range(nch):
            for dx in range(K):
                p[:, (ci * K + dx) * TILE:(ci * K + dx) * TILE + n_out] = am[ci, dx]
        return p.astype(NP_BF16)

    selp = np.zeros((128, 2 * TILE), np.float32)
    selp[:, 0:n_out] = selA
    selp[:, TILE:TILE + n_out] = selB
    selp = selp.astype(NP_BF16)

    tp = np.pad(np.asarray(target, np.float32), ((0, 0), (PAD, PAD), (0, 0)), mode="edge")
    n_cores = target.shape[1] // rows_per_core
    in_maps = []
    for i in range(n_cores):
        r0 = i * rows_per_core
        m = {"tgt": np.ascontiguousarray(tp[:, r0:r0 + rows_per_core + 2 * PAD, :]).astype(NP_BF16),
             "amatA": pack(amatA), "sel": selp}
        if tail:
            m["amatB"] = pack(amatB)
        in_maps.append(m)
    return chunks, groups, in_maps


def kernel(target, distance_kernel):
    target = np.asarray(target, np.float32)
    kern = np.asarray(distance_kernel, np.float32)
    rows_per_core = H // NCORES
    chunks, groups, in_maps = _prep_inputs(target, kern, rows_per_core, W, C)
    nc = build_nc(chunks, groups, rows_per_core, W, C)
    res = run_bass_kernel_spmd(nc, in_maps, list(range(NCORES)))
    out = np.concatenate([res.results[i]["out"] for i in range(NCORES)], axis=0)
    return out.astype(np.float32)


if __name__ == "__main__":
    from ref_np import setup_inputs_np, reference_np
    ins = setup_inputs_np()
    exp = reference_np(**ins)
    act = kernel(**ins)
    err = np.abs(act - exp).max()
    print("absmax err vs ref_np:", err)


# revision 16
# speedup vs baseline: 1.1023x; 1.1023x over previous
"""Trainium2 Bass kernel for nn_Evaluator_40870908788848 (contour-weighted loss map).

Math (matches reference.py exactly in fp32):
  fw = sum_c target[c];  b = fw > 0
  contour = max over 9x9-window *differing* neighbors of r(dy,dx),
            r = 1/(k+1e-10)  (equivalent to the reference's 1/(min_k + 1e-10))
  out = minmax_norm((fw + contour)^2) * b     (min/max global over HxW)

Device mapping (rows sharded 256/core, 4-row halo pre-padded by host):
  - The two "differing" sides (b=0: neighbors with b=1; b=1: neighbors with
    b=0) are convolutions of b resp. (1-b) with fixed tap weights -> PE-array
    banded-Toeplitz matmuls (row shifts in the stationary, column shifts in
    the moving operand's free-axis offset).  128-row interior tiles; the last
    8 output rows' cross-tile taps accumulate via a second small stationary
    reading the next row-tile, into the same PSUM bank.
  - Tap weights give each distinct r its own power-of-two exponent digit
    (descending r), so the conv sum's magnitude identifies max-r-present; a
    16-step tensor_scalar(is_ge,mult)+tensor_tensor(max) sweep decodes it
    exactly in fp32.
  - Global min/max: per-core reduce, one AllReduce(max) of [max(w), max(-w)],
    normalize + mask on device.
"""
import math
import sys

sys.path.insert(0, "/opt/trn_rl_repo")

import numpy as np

import concourse.bass as bass
import concourse.mybir as mybir
import concourse.tile as tile
import concourse.bacc as bacc
from concourse.bass_utils import run_bass_kernel_spmd

F32 = mybir.dt.float32
BF16 = mybir.dt.bfloat16
U8 = mybir.dt.uint8
OP = mybir.AluOpType
AX = mybir.AxisListType
NP_BF16 = mybir.dt.np(BF16)

H, W, C = 2048, 2048, 5
NCORES = 8
PAD = 4
K = 9
TILE = 128
COL = 512


def _plan_taps(kern):
    """Distinct r=1/(k+1e-10) descending; assign exponent digit weights."""
    kern = np.asarray(kern, np.float32)
    taps = []
    for i in range(K):
        for j in range(K):
            if i == PAD and j == PAD:
                continue
            r = np.float32(np.float32(1.0) / (kern[i, j] + np.float32(1e-10)))
            taps.append((r, i - PAD, j - PAD))
    vals = sorted({t[0] for t in taps}, reverse=True)
    chunks, cur, cur_bits, groups = [], [], 0, []
    for v in vals:
        offs = [(dy, dx) for (r, dy, dx) in taps if r == v]
        bits = max(2, math.ceil(math.log2(len(offs) + 1)) + 1)
        if cur_bits + bits > 120:
            chunks.append(cur)
            cur, cur_bits = [], 0
        w = np.float32(2.0 ** (-(cur_bits + bits)))
        cur_bits += bits
        cur.append(len(groups))
        groups.append((np.float32(v), w, offs))
    if cur:
        chunks.append(cur)
    return chunks, groups


def _stationaries(chunks, groups, n_out, tail):
    """Banded conv stationaries.

    amatA[ci,dx][p, y] = tap weight (dy=p-y-PAD) for same-tile rows;
    amatB[ci,dx][q, y] = tap weight (dy=q+TILE-PAD-y) for next-tile rows
    (used when n_out == TILE).  sel picks fw at the interior alignment.
    """
    nch = len(chunks)
    amatA = np.zeros((nch, K, 128, n_out), np.float32)
    amatB = np.zeros((nch, K, 128, n_out), np.float32)
    for ci, gids in enumerate(chunks):
        for gi in gids:
            _, wgt, offs = groups[gi]
            for (dy, dx) in offs:
                j = dx + PAD
                for y in range(n_out):
                    p = y + PAD + dy
                    if p < TILE:
                        amatA[ci, j, p, y] = wgt
                    elif tail:
                        amatB[ci, j, p - TILE, y] = wgt
    selA = np.zeros((128, n_out), np.float32)
    selB = np.zeros((128, n_out), np.float32)
    for y in range(n_out):
        p = y + PAD
        if p < TILE:
            selA[p, y] = 1.0
        elif tail:
            selB[p - TILE, y] = 1.0
    return amatA, amatB, selA, selB


def build_nc(chunks, groups, rows_per_core, width, n_chan, reps=1, no_collective=False):
    nch = len(chunks)
    n_rt = (rows_per_core + TILE - 1) // TILE
    col = min(COL, width)
    n_ct = (width + col - 1) // col
    tail = rows_per_core % TILE == 0
    pr = rows_per_core + 2 * PAD
    bt_starts = list(range(0, pr, TILE))

    nc = bacc.Bacc("TRN2", target_bir_lowering=False, debug=False)
    tgt = nc.declare_dram_parameter("tgt", [n_chan, pr, width], BF16, isOutput=False)
    amatA_d = nc.declare_dram_parameter("amatA", [128, nch * K * TILE], BF16, isOutput=False)
    sel_d = nc.declare_dram_parameter("sel", [128, 2 * TILE], BF16, isOutput=False)
    if tail:
        amatB_d = nc.declare_dram_parameter("amatB", [128, nch * K * TILE], BF16, isOutput=False)
    out_d = nc.declare_dram_parameter("out", [rows_per_core, width], F32, isOutput=True)
    scr = nc.dram_tensor("scr", [2, 128], F32)
    cc_in = nc.dram_tensor("cc_in", [1, 2], F32)
    cc_out = nc.dram_tensor("cc_out", [1, 2], F32)

    wh = width + 2 * PAD

    with tile.TileContext(nc) as tc:
        with tc.tile_pool(name="const", bufs=1) as cst, \
             tc.tile_pool(name="chp", bufs=2) as chp, \
             tc.tile_pool(name="fwp", bufs=2) as fwp, \
             tc.tile_pool(name="bt", bufs=1) as btp, \
             tc.tile_pool(name="sdp", bufs=2) as sdp, \
             tc.tile_pool(name="accp", bufs=2) as accp, \
             tc.tile_pool(name="stash", bufs=1) as stash, \
             tc.tile_pool(name="stat", bufs=1) as stat, \
             tc.tile_pool(name="ps", bufs=2, space="PSUM") as ps:

            aA_t = cst.tile([128, nch * K * TILE], BF16, tag="amatA")
            sel_t = cst.tile([128, 2 * TILE], BF16, tag="sel")
            nc.sync.dma_start(aA_t[:], amatA_d[:])
            nc.sync.dma_start(sel_t[:], sel_d[:])
            if tail:
                aB_t = cst.tile([128, nch * K * TILE], BF16, tag="amatB")
                nc.sync.dma_start(aB_t[:], amatB_d[:])

            import contextlib
            loop_cm = tc.For_i(0, reps, 1) if reps > 1 else contextlib.nullcontext()
            loop_cm.__enter__()
            gmx = stat.tile([128, 1], F32, tag="gmx")
            gnm = stat.tile([128, 1], F32, tag="gnm")
            nc.vector.memset(gmx[:], -3.0e38)
            nc.vector.memset(gnm[:], -3.0e38)

            # ---- stage 1: fw / b / bbar for every b-tile (all bf16; exact for
            # binary inputs: sums are small integers)
            b_l, bb_l, fwb_l = [], [], []
            for k, s in enumerate(bt_starts):
                L = min(TILE, pr - s)
                ch = [chp.tile([128, width], BF16, tag=f"ch{i}", name=f"ch{i}") for i in range(2)]
                fw = btp.tile([128, width], BF16, tag=f"fw{k}", name=f"fw{k}")
                nc.sync.dma_start(ch[0][:L, :], tgt[0, s:s + L, :])
                nc.sync.dma_start(ch[1][:L, :], tgt[1, s:s + L, :])
                nc.vector.tensor_tensor(fw[:L, :], ch[0][:L, :], ch[1][:L, :], OP.add)
                for cidx in range(2, n_chan):
                    cht = chp.tile([128, width], BF16, tag=f"ch{cidx % 2}")
                    nc.sync.dma_start(cht[:L, :], tgt[cidx, s:s + L, :])
                    nc.vector.tensor_tensor(fw[:L, :], fw[:L, :], cht[:L, :], OP.add)
                b_t = btp.tile([128, wh], BF16, tag=f"b{k}", name=f"b{k}")
                bb_t = btp.tile([128, wh], BF16, tag=f"bb{k}", name=f"bb{k}")
                nc.vector.tensor_scalar(b_t[:L, PAD:PAD + width], fw[:L, :], 0.0, None, OP.is_gt)
                nc.vector.tensor_scalar(bb_t[:L, PAD:PAD + width], fw[:L, :], 0.0, None, OP.is_le)
                for e in range(PAD):
                    for tt in (b_t, bb_t):
                        nc.vector.tensor_copy(tt[:L, e:e + 1], tt[:L, PAD:PAD + 1])
                        nc.vector.tensor_copy(
                            tt[:L, PAD + width + e:PAD + width + e + 1],
                            tt[:L, PAD + width - 1:PAD + width])
                b_l.append(b_t); bb_l.append(bb_t); fwb_l.append(fw)

            # ---- stage 2: conv + decode per interior row-tile
            wst_l, mst_l, nt_l = [], [], []
            for t in range(n_rt):
                r0 = t * TILE
                nt = min(TILE, rows_per_core - r0)
                Lt = min(TILE, pr - bt_starts[t])
                use_tail = tail and nt == TILE

                sdiff = [sdp.tile([128, width], F32, tag=f"sd{ci}", name=f"sd{ci}") for ci in range(nch)]
                fwis = fwp.tile([128, width], F32, tag="fwis")
                mask_t = stash.tile([128, width], U8, tag=f"mask{t}", name=f"mask{t}")

                for cc in range(n_ct):
                    c0 = cc * col
                    cw = min(col, width - c0)
                    fwi_p = ps.tile([128, col], F32, tag="fwi")
                    nc.tensor.matmul(fwi_p[:nt, :cw], sel_t[:Lt, 0:nt],
                                     fwb_l[t][:Lt, c0:c0 + cw],
                                     start=True, stop=not use_tail)
                    if use_tail:
                        nc.tensor.matmul(fwi_p[:nt, :cw], sel_t[0:8, TILE:TILE + nt],
                                         fwb_l[t + 1][0:8, c0:c0 + cw],
                                         start=False, stop=True)
                    nc.scalar.copy(fwis[:nt, c0:c0 + cw], fwi_p[:nt, :cw])
                    nc.vector.tensor_scalar(
                        mask_t[:nt, c0:c0 + cw], fwis[:nt, c0:c0 + cw], 0.0, None, OP.is_gt)
                    for ci in range(nch):
                        s1_p = ps.tile([128, col], F32, tag="s1")
                        s0_p = ps.tile([128, col], F32, tag="s0")
                        for mov, psum in ((b_l, s1_p), (bb_l, s0_p)):
                            for dx in range(K):
                                a_ap = aA_t[:Lt, (ci * K + dx) * TILE:(ci * K + dx) * TILE + nt]
                                nc.tensor.matmul(psum[:nt, :cw], a_ap,
                                                 mov[t][:Lt, c0 + dx:c0 + dx + cw],
                                                 start=(dx == 0),
                                                 stop=(dx == K - 1 and not use_tail))
                            if use_tail:
                                for dx in range(K):
                                    a_ap = aB_t[0:8, (ci * K + dx) * TILE:(ci * K + dx) * TILE + nt]
                                    nc.tensor.matmul(psum[:nt, :cw], a_ap,
                                                     mov[t + 1][0:8, c0 + dx:c0 + dx + cw],
                                                     start=False, stop=(dx == K - 1))
                        nc.vector.tensor_copy(sdiff[ci][:nt, c0:c0 + cw], s1_p[:nt, :cw])
                        nc.vector.copy_predicated(
                            sdiff[ci][:nt, c0:c0 + cw], mask_t[:nt, c0:c0 + cw],
                            s0_p[:nt, :cw])

                acc = accp.tile([128, width], F32, tag="acc")
                first = True
                for ci, gids in enumerate(chunks):
                    for gi in gids:
                        rv, wgt, _ = groups[gi]
                        dst = acc if first else accp.tile([128, width], F32, tag="cand", name="cand")
                        nc.vector.tensor_scalar(
                            dst[:nt, :], sdiff[ci][:nt, :], float(wgt), float(rv),
                            OP.is_ge, OP.mult)
                        if not first:
                            nc.vector.tensor_tensor(acc[:nt, :], acc[:nt, :], dst[:nt, :], OP.max)
                        first = False

                wt = stash.tile([128, width], F32, tag=f"w{t}", name=f"w{t}")
                nc.vector.tensor_tensor(acc[:nt, :], acc[:nt, :], fwis[:nt, :], OP.add)
                nc.scalar.square(wt[:nt, :], acc[:nt, :])
                lred = stat.tile([128, 2], F32, tag="lred")
                nc.vector.tensor_reduce(lred[:nt, 0:1], wt[:nt, :], AX.X, OP.max)
                nc.vector.tensor_reduce(lred[:nt, 1:2], wt[:nt, :], AX.X, OP.min)
                nc.vector.tensor_tensor(gmx[:nt, :], gmx[:nt, :], lred[:nt, 0:1], OP.max)
                nc.vector.scalar_tensor_tensor(
                    gnm[:nt, :], lred[:nt, 1:2], -1.0, gnm[:nt, :], OP.mult, OP.max)
                wst_l.append(wt); mst_l.append(mask_t); nt_l.append(nt)

            loop_cm.__exit__(None, None, None)
            # ---- global min/max via DMA round-trip + AllReduce(max)
            nc.sync.dma_start(scr[0, :], gmx[:, 0:1])
            nc.sync.dma_start(scr[1, :], gnm[:, 0:1])
            srow = stat.tile([1, 2, 128], F32, tag="srow")
            nc.sync.dma_start(srow[0:1, :, :], scr[:, :])
            loc = stat.tile([1, 2], F32, tag="loc")
            nc.vector.tensor_reduce(loc[0:1, :], srow[0:1, :, :], AX.X, OP.max)
            nc.sync.dma_start(cc_in[:], loc[0:1, :])
            if no_collective:
                nc.sync.dma_start(cc_out[:], cc_in[:])
            else:
                nc.gpsimd.collective_compute(
                    "AllReduce", OP.max, replica_groups=[list(range(NCORES))],
                    ins=[cc_in[:]], outs=[cc_out[:]])
            st = stat.tile([1, 2], F32, tag="st")
            nc.sync.dma_start(st[:], cc_out[:])
            bcs = stat.tile([1, 2], F32, tag="bcs")
            rngt = stat.tile([1, 1], F32, tag="rngt")
            nc.vector.tensor_tensor(rngt[0:1, :], st[0:1, 0:1], st[0:1, 1:2], OP.add)
            nc.vector.tensor_scalar(rngt[0:1, :], rngt[0:1, :], 1e-10, None, OP.add)
            nc.vector.reciprocal(bcs[0:1, 0:1], rngt[0:1, :])
            mnt = stat.tile([1, 1], F32, tag="mnt")
            nc.vector.tensor_scalar(mnt[0:1, :], st[0:1, 1:2], -1.0, None, OP.mult)
            nc.vector.tensor_tensor(bcs[0:1, 1:2], mnt[0:1, :], bcs[0:1, 0:1], OP.mult)
            ones1 = stat.tile([1, 128], F32, tag="ones1")
            nc.vector.memset(ones1[:], 1.0)
            pbc = ps.tile([128, 2], F32, tag="pbc")
            nc.tensor.matmul(pbc[:], ones1[0:1, :], bcs[0:1, :], start=True, stop=True)
            bct = stat.tile([128, 2], F32, tag="bct")
            nc.vector.tensor_copy(bct[:], pbc[:])

            # ---- pass 2: normalize + mask + store
            for t in range(n_rt):
                nt = nt_l[t]
                r0 = t * TILE
                ot = accp.tile([128, width], F32, tag="cand", name="ot")
                nc.vector.tensor_scalar(
                    ot[:nt, :], wst_l[t][:nt, :], bct[:nt, 0:1], bct[:nt, 1:2],
                    OP.mult, OP.subtract)
                nc.vector.tensor_tensor(ot[:nt, :], ot[:nt, :], mst_l[t][:nt, :], OP.mult)
                nc.sync.dma_start(out_d[r0:r0 + nt, :], ot[:nt, :])
    nc.finalize()
    return nc


def _prep_inputs(target, kern, rows_per_core, width, n_chan):
    chunks, groups = _plan_taps(kern)
    nch = len(chunks)
    tail = rows_per_core % TILE == 0
    n_out = min(TILE, rows_per_core)
    amatA, amatB, selA, selB = _stationaries(chunks, groups, n_out, tail)

    def pack(am):
        p = np.zeros((128, nch * K * TILE), np.float32)
        for ci in range(nch):
            for dx in range(K):
                p[:, (ci * K + dx) * TILE:(ci * K + dx) * TILE + n_out] = am[ci, dx]
        return p.astype(NP_BF16)

    selp = np.zeros((128, 2 * TILE), np.float32)
    selp[:, 0:n_out] = selA
    selp[:, TILE:TILE + n_out] = selB
    selp = selp.astype(NP_BF16)

    tp = np.pad(np.asarray(target, np.float32), ((0, 0), (PAD, PAD), (0, 0)), mode="edge")
    n_cores = target.shape[1] // rows_per_core
    in_maps = []
    for i in range(n_cores):
        r0 = i * rows_per_core
        m = {"tgt": np.ascontiguousarray(tp[:, r0:r0 + rows_per_core + 2 * PAD, :]).astype(NP_BF16),
             "amatA": pack(amatA), "sel": selp}
        if tail:
            m["amatB"] = pack(amatB)
        in_maps.append(m)
    return chunks, groups, in_maps


def kernel(target, distance_kernel):
    target = np.asarray(target, np.float32)
    kern = np.asarray(distance_kernel, np.float32)
    rows_per_core = H // NCORES
    chunks, groups, in_maps = _prep_inputs(target, kern, rows_per_core, W, C)
    nc = build_nc(chunks, groups, rows_per_core, W, C)
    res = run_bass_kernel_spmd(nc, in_maps, list(range(NCORES)))
    out = np.concatenate([res.results[i]["out"] for i in range(NCORES)], axis=0)
    return out.astype(np.float32)


if __name__ == "__main__":
    from ref_np import setup_inputs_np, reference_np
    ins = setup_inputs_np()
    exp = reference_np(**ins)
    act = kernel(**ins)
    err = np.abs(act - exp).max()
    print("absmax err vs ref_np:", err)
